# revision 14
# baseline (speedup 1.0000x reference)
"""BigBird attention (faithful .view-split variant) on 8 Trainium2 NeuronCores.

Sharding: the reference's `.reshape(B, H, S, hd)` head-split makes each
(batch, head) attend over a [2048, 64] row-major reshape of a 128-token
chunk's [128, 1024] projection. The 2*16 = 32 (b,h) pairs are sharded 4 per
core (batch x head parallel). The output projection is computed per-core as
a partial sum over its 4 heads (row-parallel over Wo), partials are summed
on the host.

Per core (v2 — tuned for PE occupancy / HAM warm clock):
  A) QKV projections in bf16 (weights + x cast on host), per-kt weight
     tiles so compute starts after the first slice lands. PSUM [128,1024]
     tiles (2 banks); copies to the DRAM bounce run on ScalarE.
  B) Per chunk: block-sparse attention. Score strips are packed into
     [128,1024] PSUM group tiles (4 strips of 256 per group / 2 glob
     halves), ONE wide exp per group on ScalarE (scale=1/8 folded),
     inactive blocks zeroed on VectorE. AV matmuls in bf16 with a ones
     column on V giving softmax sums for free. Group pipeline: scores of
     group g+2 are emitted before AV of group g so the PE never stalls on
     exp. Normalization per psum bank as soon as its last AV lands:
     reciprocal straight off the psum sums row, one DRAM-broadcast hop,
     multiply on VectorE.
  C) Partial output projection y^T = sum_h Wo_h O_h^T with head pairs
     stacked on partitions (K=128, bf16), bf16 partials, copies split
     Vector/Scalar, DMA'd out per tile.

The block mask (band + global cols 0/31 + 3 random blocks) is known at
trace time from src_blocks/tgt_blocks, so the sparsity plan is specialized
per call.
"""

import numpy as np
import ml_dtypes

import concourse.bass as bass
import concourse.mybir as mybir
import concourse.tile as tile
from concourse import bacc
from concourse.bass_utils import run_bass_kernel_spmd

B, S, DIM = 2, 2048, 1024
NHEADS, HD, BLK = 16, 64, 64
NB = S // BLK          # 32 block rows/cols
NCORES = 8
HPC = NHEADS * B // NCORES  # 4 chunks (b,h) per core
P = 128

f32 = mybir.dt.float32
bf16 = mybir.dt.bfloat16

LAST_EXEC_NS = None
LAST_TRACE = None


def _block_mask(src_blocks, tgt_blocks):
    i = np.arange(NB)[:, None]
    j = np.arange(NB)[None, :]
    bm = (np.abs(i - j) <= 1) | (j == 0) | (j == NB - 1)
    bm[np.asarray(src_blocks), np.asarray(tgt_blocks)] = True
    return bm


def _plan_strips(bm):
    """Cover the active blocks with k-stacked strips.

    Strip = dict(k=[kb...] (1 or 2 k-blocks stacked on partitions),
                 q0, qn (q-block run), act [len(k), qn] bool, kind).
    Active cells are claimed exactly once across strips so softmax sums
    are exact.  The glob strip (columns 0/31, all q) is implicit — it is
    handled separately; this returns band + extra strips only.
    """
    claimed = np.zeros((NB, NB), bool)
    claimed[:, 0] = True
    claimed[:, NB - 1] = True
    strips = []
    # band strips: k-pair (2m-1, 2m), q-blocks [2m-2, 2m+2)
    for m in range(1, NB // 2):
        kbs = [2 * m - 1, 2 * m]
        q0, qn = 2 * m - 2, 4
        act = np.zeros((2, qn), bool)
        for ki, k in enumerate(kbs):
            for qi in range(qn):
                q = q0 + qi
                if bm[q, k] and not claimed[q, k]:
                    act[ki, qi] = True
                    claimed[q, k] = True
        strips.append(dict(k=kbs, q0=q0, qn=qn, act=act, kind="band"))
    # leftover random blocks
    rem = np.argwhere(bm & ~claimed)
    byk = {}
    for q, k in rem:
        byk.setdefault(int(k), []).append(int(q))
    for k, qs in sorted(byk.items()):
        qs = sorted(qs)
        while qs:
            q0 = min(max(qs[0] - 1, 0), NB - 4)
            qn = 4
            act = np.zeros((1, qn), bool)
            rest = []
            for q in qs:
                if q0 <= q < q0 + qn:
                    act[0, q - q0] = True
                    claimed[q, k] = True
                else:
                    rest.append(q)
            qs = rest
            strips.append(dict(k=[k], q0=q0, qn=qn, act=act, kind="extra"))
    return strips


def _plan_groups(strips):
    """Pack strips into [128, <=1024] psum group tiles.

    Group kinds: "glob" (one 1024-wide half of the global columns) and
    "strip" (up to 4 band/extra strips at 256 cols each; bands and extras
    are never mixed within one group so every matmul into a given psum
    bank writes the same partition range).
    """
    groups = [dict(kind="glob", qh=0), dict(kind="glob", qh=1)]
    bands = sorted([s for s in strips if s["kind"] == "band"],
                   key=lambda s: s["q0"])
    extras = sorted([s for s in strips if s["kind"] == "extra"],
                    key=lambda s: s["q0"])
    for j in range(0, len(bands), 4):
        groups.append(dict(kind="strip", strips=bands[j:j + 4]))
    for j in range(0, len(extras), 4):
        groups.append(dict(kind="strip", strips=extras[j:j + 4]))
    return groups


def _build_program(strips, use_bias=True):
    nc = bacc.Bacc("TRN2", target_bir_lowering=False, debug=False,
                   num_devices=NCORES)

    # ---- per-core external inputs ----
    d_xt = nc.dram_tensor("xt", [HPC, P, DIM], bf16, kind="ExternalInput")
    d_wq = nc.dram_tensor("wq", [P, 8 * DIM], bf16, kind="ExternalInput")
    d_wk = nc.dram_tensor("wk", [P, 8 * DIM], bf16, kind="ExternalInput")
    d_wv = nc.dram_tensor("wv", [P, 8 * DIM], bf16, kind="ExternalInput")
    d_bq = nc.dram_tensor("bq", [1, DIM], f32, kind="ExternalInput")
    d_bk = nc.dram_tensor("bk", [1, DIM], f32, kind="ExternalInput")
    d_bv = nc.dram_tensor("bv", [1, DIM], f32, kind="ExternalInput")
    d_wo = nc.dram_tensor("wo", [2, P, DIM], bf16, kind="ExternalInput")
    # y^T partials, tiled: [qb, m2, p, hh*512+c] = y^T[(2*m2+hh)*128+p,
    # qb*512+c] so each phase-C psum tile flushes with ONE dma
    d_yt = nc.dram_tensor("yt", [4, 4, P, 1024], bf16, kind="ExternalOutput")

    with tile.TileContext(nc) as tc:
        _emit(nc, tc, strips, d_xt, (d_wq, d_wk, d_wv),
              (d_bq, d_bk, d_bv), d_wo, d_yt, use_bias)
    nc.compile()
    return nc


def _emit(nc, tc, strips, d_xt, d_w, d_b, d_wo, d_yt, use_bias):
    from contextlib import ExitStack
    groups = _plan_groups(strips)
    with ExitStack() as ctx:
        psMM = ctx.enter_context(tc.tile_pool(name="psMM", bufs=2,
                                              space="PSUM"))
        psOT = ctx.enter_context(tc.tile_pool(name="psOT", bufs=4,
                                              space="PSUM"))
        dram = ctx.enter_context(tc.tile_pool(name="dram", bufs=1,
                                              space="DRAM"))
        sbB = ctx.enter_context(tc.tile_pool(name="sbB", bufs=1))
        sbQK = ctx.enter_context(tc.tile_pool(name="sbQK", bufs=1))
        sbV = ctx.enter_context(tc.tile_pool(name="sbV", bufs=1))
        sbN = ctx.enter_context(tc.tile_pool(name="sbN", bufs=2))
        lp = ctx.enter_context(tc.tile_pool(name="lp", bufs=2))

        # DRAM scratch: per-chunk projection bounces
        dlin = {}
        for nm, shp in (("q", [S, P]), ("k", [S, P]), ("v", [P, DIM])):
            dlin[nm] = [dram.tile(shp, bf16, tag=f"d{nm}{i}",
                                  name=f"d{nm}{i}")
                        for i in range(HPC)]

        # Issue order matters: the SP queue gets what phase A needs first
        # (x0 + the 8 Wq slices); the idle Pool queue issues the rest so
        # HWDGE fixed overhead (~625ns/DMA) doesn't serialize startup.
        xtiles = [sbB.tile([P, DIM], bf16, tag=f"xt{i}", name=f"xt{i}")
                  for i in range(HPC)]
        nc.sync.dma_start(xtiles[0][:], d_xt[0])
        wt = {}
        for nm, dw in zip("qkv", d_w):
            wt[nm] = [sbB.tile([P, DIM], bf16, tag=f"w{nm}{kt}",
                               name=f"w{nm}{kt}") for kt in range(8)]
        for kt in range(8):
            nc.sync.dma_start(wt["q"][kt][:],
                              d_w[0][:, kt * DIM:(kt + 1) * DIM])
        for i in range(1, HPC):
            nc.sync.dma_start(xtiles[i][:], d_xt[i])
        for nm, dw in (("k", d_w[1]), ("v", d_w[2])):
            for kt in range(8):
                nc.gpsimd.dma_start(wt[nm][kt][:],
                                    dw[:, kt * DIM:(kt + 1) * DIM])
        wob = sbB.tile([P, 2 * DIM], bf16, tag="wob")
        nc.gpsimd.dma_start(wob[:, 0:DIM], d_wo[0])
        nc.gpsimd.dma_start(wob[:, DIM:2 * DIM], d_wo[1])

        # Preload the exp ACT table with a dummy activation (overlaps A).
        warm = sbB.tile([1, 8], f32, tag="warm")
        nc.vector.memset(warm[:], 0.0)
        nc.scalar.activation(warm[:], warm[:],
                             mybir.ActivationFunctionType.Exp, scale=1.0)

        if use_bias:
            bts = {}
            for nm, db in zip("qkv", d_b):
                bts[nm] = sbB.tile([P, DIM], f32, tag=f"b{nm}")
                nc.gpsimd.dma_start(bts[nm][:], db[:].to_broadcast((P, DIM)))

        # O2 tiles: head-pair-stacked normalized O^T, consumed by phase C
        o2 = [sbB.tile([P, S], bf16, tag=f"o2_{a}", name=f"o2_{a}")
              for a in range(2)]

        # per-chunk attention-input tiles (filled during phase A)
        qts, kts, ktgs, v2bs, v2gs, vxss = [], [], [], [], [], []

        # ---------------- Phase A: QKV projections (chunk-major) ----------
        for i in range(HPC):
            for nm in "qkv":
                ps = psMM.tile([P, 1024], f32, tag="mm")
                for half in range(2):
                    for kt in range(8):
                        nc.tensor.matmul(
                            ps[:, half * 512:(half + 1) * 512],
                            lhsT=xtiles[i][:, kt * P:(kt + 1) * P],
                            rhs=wt[nm][kt][:, half * 512:(half + 1) * 512],
                            start=(kt == 0), stop=(kt == 7))
                if nm == "v":
                    lint = lp.tile([P, DIM], bf16, tag="linv", name="lintv")
                    for half in range(2):
                        sl = slice(half * 512, (half + 1) * 512)
                        if use_bias:
                            nc.vector.tensor_add(
                                lint[:, sl].rearrange("p (c d) -> p c d", d=64),
                                ps[:, sl].rearrange("p (c d) -> p c d", d=64),
                                bts[nm][:, sl].rearrange("p (c d) -> p c d", d=64))
                        else:
                            nc.vector.tensor_copy(
                                lint[:, sl].rearrange("p (c d) -> p c d", d=64),
                                ps[:, sl].rearrange("p (c d) -> p c d", d=64))
                else:
                    # d-axis padded to 128 (zeros) so the bounce is DMA-
                    # transposable: dram layout [s', 128] = [t, (c, d|pad)]
                    lint = lp.tile([P, 2 * DIM], bf16, tag=f"lin{nm}",
                                   name=f"lint{nm}")
                    nc.vector.memset(
                        lint[:].rearrange("p (c x) -> p c x",
                                          x=P)[:, :, 64:P], 0.0)
                    for half in range(2):
                        out_ap = lint[:].rearrange(
                            "p (c x) -> p c x",
                            x=P)[:, half * 8:(half + 1) * 8, 0:64]
                        in_ap = ps[:, half * 512:(half + 1) * 512
                                   ].rearrange("p (c d) -> p c d", d=64)
                        if use_bias:
                            nc.vector.tensor_add(
                                out_ap, in_ap,
                                bts[nm][:, half * 512:(half + 1) * 512
                                        ].rearrange("p (c d) -> p c d", d=64))
                        else:
                            nc.vector.tensor_copy(out_ap, in_ap)
                nc.gpsimd.dma_start(dlin[nm][i][:], lint[:])
                # transposes chase each bounce (SP queue)
                if nm == "q":
                    qt = sbQK.tile([P, S], bf16, tag=f"qt{i}", name=f"qt{i}")
                    nc.sync.dma_start(qt[:], dlin["q"][i][:], transpose=True)
                elif nm == "k":
                    kt_ = sbQK.tile([P, S], bf16, tag=f"kt{i}",
                                    name=f"kt{i}")
                    nc.sync.dma_start(kt_[:], dlin["k"][i][:], transpose=True)
                    ktg = sbV.tile([P, P], bf16, tag=f"ktg{i}",
                                   name=f"ktg{i}")
                    nc.sync.dma_start(ktg[:, 0:64], dlin["k"][i][0:64],
                                      transpose=True)
                    nc.sync.dma_start(ktg[:, 64:128], dlin["k"][i][S - 64:S],
                                      transpose=True)

            # V in band-pair layout: group g <-> k-blocks (2g+1, 2g+2)
            v2b = sbV.tile([P, 15 * 65], bf16, tag=f"v2b{i}", name=f"v2b{i}")
            nc.gpsimd.dma_start(
                v2b[:].rearrange("p (g e) -> p g e", e=65)[:, :, 0:64],
                dlin["v"][i][4:124].rearrange("(g a) (b d) -> (a b) g d",
                                              a=8, d=64))
            nc.vector.memset(
                v2b[:].rearrange("p (g e) -> p g e", e=65)[:, :, 64:65], 1.0)
            # V glob pair: rows 0:64 = block 0, 64:128 = block 31, + ones col
            v2g = sbV.tile([P, 65], bf16, tag=f"v2g{i}", name=f"v2g{i}")
            nc.gpsimd.dma_start(
                v2g[0:64, 0:64],
                dlin["v"][i][0:4].rearrange("t (c d) -> (t c) d", d=64))
            nc.gpsimd.dma_start(
                v2g[64:128, 0:64],
                dlin["v"][i][124:128].rearrange("t (c d) -> (t c) d", d=64))
            nc.vector.memset(v2g[:, 64:65], 1.0)
            # V tiles for extra strips
            vxs = {}
            for si, st in enumerate(s for s in strips if s["kind"] == "extra"):
                kb = st["k"][0]
                vx = sbV.tile([64, 65], bf16, tag=f"vx{i}_{si}",
                              name=f"vx{i}_{si}")
                nc.gpsimd.dma_start(
                    vx[:, 0:64],
                    dlin["v"][i][kb * 4:kb * 4 + 4].rearrange(
                        "t (c d) -> (t c) d", d=64))
                nc.vector.memset(vx[:, 64:65], 1.0)
                vxs[id(st)] = vx
            qts.append(qt)
            kts.append(kt_)
            ktgs.append(ktg)
            v2bs.append(v2b)
            v2gs.append(v2g)
            vxss.append(vxs)

        # ---------------- Phase B: attention per chunk ----------------
        for i in range(HPC):
            qt, kt_, ktg = qts[i], kts[i], ktgs[i]
            v2b, v2g, vxs = v2bs[i], v2gs[i], vxss[i]

            ot_h = [psOT.tile([65, 512], f32, tag="ot",
                              name=f"ot{i}_{h}") for h in range(4)]
            # count AV pieces per psum bank so stop flags + normalize
            # emission land on the last piece into each bank
            npieces = [0] * 4
            all_q_spans = [(0, S), ]  # glob strip covers everything
            for st in strips:
                all_q_spans.append((st["q0"] * BLK,
                                    (st["q0"] + st["qn"]) * BLK))
            for qlo, qhi in all_q_spans:
                q = qlo
                while q < qhi:
                    bk2 = q // 512
                    qe = min(qhi, (bk2 + 1) * 512)
                    npieces[bk2] += 1
                    q = qe
            done = [0] * 4
            norm_emitted = [False] * 4
            # per-chunk sums staging: bank h sums land in cols h*512..
            srows = sbN.tile([65, S], f32, tag="srows", name=f"srows{i}", bufs=1)

            def emit_norm(h):
                # stage bank h's sums row; the reciprocal + broadcast runs
                # once per chunk (multi-lane) when the last bank lands
                nc.scalar.copy(srows[64:65, h * 512:(h + 1) * 512],
                               ot_h[h][64:65, :])
                if not all(norm_emitted):
                    return
                dsum = dram.tile([1, S], f32, tag=f"dsum{i % 2}",
                                 name=f"dsum{i % 2}")
                nc.sync.dma_start(dsum[:], srows[64:65, :])
                ssum = sbN.tile([P, 16], f32, tag="ssum", name=f"ssum{i}")
                nc.sync.dma_start(
                    ssum[:], dsum[:].rearrange("o (p f) -> (o p) f", f=16))
                rr = sbN.tile([P, 16], f32, tag="rr", name=f"rr{i}")
                nc.vector.reciprocal(rr[:], ssum[:])
                drr = dram.tile([1, S], f32, tag=f"drr{i % 2}",
                                name=f"drr{i % 2}")
                nc.sync.dma_start(
                    drr[:].rearrange("o (p f) -> (o p) f", f=16), rr[:])
                rbc = sbN.tile([64, S], f32, tag="rbc", name=f"rbc{i}", bufs=1)
                nc.sync.dma_start(rbc[:], drr[:].to_broadcast((64, S)))
                a, half = i // 2, i % 2
                if half == 0:
                    for h2 in range(4):
                        nc.vector.tensor_mul(
                            o2[a][0:64, h2 * 512:(h2 + 1) * 512],
                            ot_h[h2][0:64, :],
                            rbc[:, h2 * 512:(h2 + 1) * 512])
                else:
                    o2t = sbN.tile([64, S], bf16, tag="o2t", name=f"o2t{i}", bufs=1)
                    for h2 in range(4):
                        nc.vector.tensor_mul(
                            o2t[:, h2 * 512:(h2 + 1) * 512],
                            ot_h[h2][0:64, :],
                            rbc[:, h2 * 512:(h2 + 1) * 512])
                    nc.sync.dma_start(o2[a][64:128, :], o2t[:])

            def av_pieces(qlo, qhi, lhs, et, et_col0, rows):
                # et covers q range [qlo, qhi) starting at column et_col0
                q = qlo
                while q < qhi:
                    bk2 = q // 512
                    qe = min(qhi, (bk2 + 1) * 512)
                    nc.tensor.matmul(
                        ot_h[bk2][0:65, q - bk2 * 512:qe - bk2 * 512],
                        lhsT=lhs,
                        rhs=et[0:rows, et_col0 + q - qlo:et_col0 + qe - qlo],
                        start=(done[bk2] == 0),
                        stop=(done[bk2] == npieces[bk2] - 1))
                    done[bk2] += 1
                    if done[bk2] == npieces[bk2] and not norm_emitted[bk2]:
                        norm_emitted[bk2] = True
                        emit_norm(bk2)
                    q = qe

            with tc.tile_pool(name=f"pe{i}", bufs=1) as pe:
                # --- pipelined group loop: scores g+2 ahead of AV g ---
                sc_ps = [None] * len(groups)
                em_t = [None] * len(groups)

                def emit_scores(gi):
                    g = groups[gi]
                    ps = psMM.tile([P, 1024], f32, tag="mm")
                    if g["kind"] == "glob":
                        for half in range(2):
                            c0 = g["qh"] * 1024 + half * 512
                            nc.tensor.matmul(
                                ps[:, half * 512:(half + 1) * 512],
                                lhsT=ktg[0:64, :],
                                rhs=qt[0:64, c0:c0 + 512],
                                start=True, stop=True)
                    else:
                        # per 512-bank: one accumulation group over the
                        # (disjoint-column) strips that land in it
                        by_bank = {}
                        for idx, st in enumerate(g["strips"]):
                            by_bank.setdefault(idx // 2, []).append((idx, st))
                        for bank, items in by_bank.items():
                            for j, (idx, st) in enumerate(items):
                                col = idx * 256
                                qlo = st["q0"] * BLK
                                qn = st["qn"] * BLK
                                if st["kind"] == "band":
                                    k0 = st["k"][0] * BLK
                                    nc.tensor.matmul(
                                        ps[:, col:col + qn],
                                        lhsT=kt_[0:64, k0:k0 + 128],
                                        rhs=qt[0:64, qlo:qlo + qn],
                                        start=(j == 0),
                                        stop=(j == len(items) - 1))
                                else:
                                    kb = st["k"][0]
                                    nc.tensor.matmul(
                                        ps[0:64, col:col + qn],
                                        lhsT=kt_[0:64,
                                                 kb * BLK:kb * BLK + 64],
                                        rhs=qt[0:64, qlo:qlo + qn],
                                        start=(j == 0),
                                        stop=(j == len(items) - 1))
                    sc_ps[gi] = ps

                def emit_exp(gi):
                    g = groups[gi]
                    ps = sc_ps[gi]
                    if g["kind"] == "glob":
                        et = pe.tile([P, 1024], bf16, tag=f"eg{gi}",
                                     name=f"eg{i}_{gi}")
                        nc.scalar.activation(
                            et[:], ps[:],
                            mybir.ActivationFunctionType.Exp, scale=0.125)
                    else:
                        width = 256 * len(g["strips"])
                        rows = P if g["strips"][0]["kind"] == "band" else 64
                        et = pe.tile([P, 1024], bf16, tag=f"eg{gi}",
                                     name=f"eg{i}_{gi}")
                        nc.scalar.activation(
                            et[0:rows, 0:width], ps[0:rows, 0:width],
                            mybir.ActivationFunctionType.Exp, scale=0.125)
                        # zero inactive blocks
                        for idx, st in enumerate(g["strips"]):
                            col = idx * 256
                            for ki in range(len(st["k"])):
                                for qi in range(st["qn"]):
                                    if not st["act"][ki, qi]:
                                        nc.vector.memset(
                                            et[ki * 64:(ki + 1) * 64,
                                               col + qi * 64:
                                               col + (qi + 1) * 64], 0.0)
                    em_t[gi] = et

                def emit_av(gi):
                    g = groups[gi]
                    et = em_t[gi]
                    if g["kind"] == "glob":
                        q0 = g["qh"] * 1024
                        av_pieces(q0, q0 + 1024, v2g[:], et, 0, 128)
                    else:
                        for idx, st in enumerate(g["strips"]):
                            col = idx * 256
                            qlo = st["q0"] * BLK
                            qhi = (st["q0"] + st["qn"]) * BLK
                            if st["kind"] == "band":
                                gidx = (st["k"][0] - 1) // 2
                                av_pieces(qlo, qhi,
                                          v2b[:, gidx * 65:(gidx + 1) * 65],
                                          et, col, 128)
                            else:
                                av_pieces(qlo, qhi, vxs[id(st)][:],
                                          et, col, 64)

                ng = len(groups)
                emit_scores(0)
                if ng > 1:
                    emit_scores(1)
                for gi in range(ng):
                    emit_exp(gi)
                    if gi + 2 < ng:
                        emit_scores(gi + 2)
                    emit_av(gi)

        # ---------------- Phase C: partial output projection --------------
        with tc.tile_pool(name="yp", bufs=3) as yp:
            for qb in range(4):
                for m2 in range(4):
                    ps = psMM.tile([P, 1024], f32, tag="mm")
                    for hh in range(2):
                        mt = 2 * m2 + hh
                        for a in range(2):
                            nc.tensor.matmul(
                                ps[:, hh * 512:(hh + 1) * 512],
                                lhsT=wob[:, a * DIM + mt * P:
                                         a * DIM + (mt + 1) * P],
                                rhs=o2[a][:, qb * 512:(qb + 1) * 512],
                                start=(a == 0), stop=(a == 1))
                    yt = yp.tile([P, 1024], bf16, tag="yt")
                    nc.vector.tensor_copy(yt[:, 0:512], ps[:, 0:512])
                    nc.scalar.copy(yt[:, 512:1024], ps[:, 512:1024])
                    nc.sync.dma_start(d_yt[qb, m2], yt[:])


def kernel(x, Wq, bq, Wk, bk, Wv, bv, Wo, bo, src_blocks, tgt_blocks,
           _trace=False):
    global LAST_EXEC_NS, LAST_TRACE
    x = np.asarray(x, np.float32)
    bm = _block_mask(np.asarray(src_blocks), np.asarray(tgt_blocks))
    strips = _plan_strips(bm)
    use_bias = bool(np.any(np.asarray(bq)) or np.any(np.asarray(bk))
                    or np.any(np.asarray(bv)))
    nc = _build_program(strips, use_bias)

    # host-side shard prep
    # W layout for rhs: w[p, kt*1024 + j] = W[j, kt*128 + p]
    def w_rhs(W):
        Wt = np.ascontiguousarray(np.asarray(W, np.float32).T)  # [in, out]
        return np.ascontiguousarray(
            Wt.reshape(8, P, DIM).transpose(1, 0, 2).reshape(P, 8 * DIM)
        ).astype(ml_dtypes.bfloat16)

    wq_h, wk_h, wv_h = w_rhs(Wq), w_rhs(Wk), w_rhs(Wv)
    WoT = np.asarray(Wo, np.float32).T  # [in(=64*head), out]
    x4 = x.reshape(B, NHEADS, P, DIM)

    in_maps = []
    for c in range(NCORES):
        b = c // 4
        h0 = 4 * (c % 4)
        xc = x4[b, h0:h0 + 4]                       # [4, 128, 1024]
        xt = np.ascontiguousarray(xc.transpose(0, 2, 1))  # [4, 1024, 128]
        # xt dram layout [4, 128, 8*128]: xts[i, p, kt*128+t] = x[t, kt*128+p]
        xts = np.ascontiguousarray(
            xt.reshape(HPC, 8, P, P).transpose(0, 2, 1, 3).reshape(
                HPC, P, 8 * P)).astype(ml_dtypes.bfloat16)
        wo_c = np.zeros((2, P, DIM), ml_dtypes.bfloat16)
        for a in range(2):
            r0 = 64 * (h0 + 2 * a)
            wo_c[a] = WoT[r0:r0 + 128].astype(ml_dtypes.bfloat16)
        in_maps.append({
            "xt": xts,
            "wq": wq_h, "wk": wk_h, "wv": wv_h,
            "bq": np.asarray(bq, np.float32).reshape(1, DIM),
            "bk": np.asarray(bk, np.float32).reshape(1, DIM),
            "bv": np.asarray(bv, np.float32).reshape(1, DIM),
            "wo": wo_c,
        })

    res = run_bass_kernel_spmd(nc, in_maps, core_ids=list(range(NCORES)),
                               trace=_trace)
    LAST_EXEC_NS = res.exec_time_ns
    LAST_TRACE = (res.instructions_and_trace[1]
                  if res.instructions_and_trace else None)

    y = np.zeros((B, S, DIM), np.float32)
    for c in range(NCORES):
        yt_r = np.asarray(res.results[c]["yt"])  # [qb, m2, p, hh*512+c]
        yT = yt_r.reshape(4, 4, P, 2, 512).transpose(1, 3, 2, 0, 4
                                                     ).reshape(DIM, S)
        y[c // 4] += yT.T.astype(np.float32)
    y += np.asarray(bo, np.float32)
    return y
